# revision 4
# baseline (speedup 1.0000x reference)
"""CQAttention (BiDAF-style context-query attention) on 8 TRN2 NeuronCores.

Full shapes: contex [64, 512, 256], question [64, 64, 256],
W_weight [1, 768], W_bias [1] -> out [64, 512, 1024].

Sharding: pure data-parallel over batch, 8 batches per core.

Math notes (per batch, C=[512,256], Q=[64,256], w=[wq|wc|wi]):
  S[i,j] = sum_d C[i,d]*wi[d]*Q[j,d] + C[i].wc + Q[j].wq + b
  S1 = softmax_j(S), S2 = softmax_i(S)
  - b drops out of both softmaxes; s_c drops out of S1; s_q drops out of S2.
  - E1 = exp(s_i + s_q[j]), r1[i] = sum_j E1;  S1 = E1/r1
  - E2 = exp(s_i + s_c[i]), r2[j] = sum_i E2;  S2 = E2/r2
  - A  = S1 @ Q = (E1 @ Q)/r1
  - Bm = (S1 @ S2^T) @ C = S1 @ (S2^T @ C) = (E1 @ C2)/r1, C2 = (E2^T @ C)/r2
  r1/r2 are obtained for free as ones-columns appended to the matmul rhs.
  out = [C | A | C*A | C*Bm]

DMA design:
  - context rows are mapped i = 4p + t (partition-major), so each partition
    owns 4 consecutive DRAM rows: C loads move 4KB-contiguous lines and the
    merged output store moves 16KB-contiguous lines per partition.
  - ALL input DMAs are issued up front (before any compute is emitted) into
    persistent tiles, so no dma_start ever queues behind compute on its
    issuing engine.  The first loads are split across both HWDGE rings.
  - The four output blocks of a batch are assembled in one [128, 4, 1024]
    staging tile and shipped as a single 2MB store on the sync ring
    (~330 GB/s observed vs ~270 GB/s for the old 1KB-line stores).

Emission is software-pipelined: phase A (casts/PE transposes) runs
LOOKAHEAD batches ahead of phase B (main matmul chain) so the in-order PE
stream always has independent transpose work, and each batch's single big
store overlaps the following batches' compute.
"""

import numpy as np

B, LC, LQ, D = 64, 512, 64, 256
NCORES = 8
BL = B // NCORES  # batches per core
LOOKAHEAD = 3
NSLOT = LOOKAHEAD + 1

_NC_CACHE = None


def _build_nc():
    import concourse.bass as bass
    import concourse.mybir as mybir
    from concourse import bacc
    from concourse import masks
    from concourse import tile
    from contextlib import ExitStack

    f32 = mybir.dt.float32
    bf16 = mybir.dt.bfloat16
    AF = mybir.ActivationFunctionType
    MUL = mybir.AluOpType.mult
    ts = bass.ts

    nc = bacc.Bacc("TRN2", target_bir_lowering=False, debug=False)
    C_d = nc.dram_tensor("contex", [BL, LC, D], f32, kind="ExternalInput")
    Q_d = nc.dram_tensor("question", [BL, LQ, D], f32, kind="ExternalInput")
    W_d = nc.dram_tensor("W_weight", [1, 3 * D], f32, kind="ExternalInput")
    out_d = nc.dram_tensor("out", [BL, LC, 4 * D], f32, kind="ExternalOutput")

    with tile.TileContext(nc) as tc, ExitStack() as ctx:
        const = ctx.enter_context(tc.tile_pool(name="const", bufs=1))
        sb = ctx.enter_context(tc.tile_pool(name="sb", bufs=NSLOT))
        stg = ctx.enter_context(tc.tile_pool(name="stg", bufs=NSLOT))
        ps_tc = ctx.enter_context(tc.tile_pool(name="ps_tc", bufs=2, space="PSUM"))
        ps_si = ctx.enter_context(tc.tile_pool(name="ps_si", bufs=2, space="PSUM"))
        ps_mm = ctx.enter_context(tc.tile_pool(name="ps_mm", bufs=4, space="PSUM"))

        # ---- all input DMAs, issued before any compute exists ----
        # sync ring: weights + Q + first C batch (stores only start ~11us in)
        W_sb2 = const.tile([1, 2, D], f32, tag="W_sb2")
        nc.sync.dma_start(W_sb2[:, 0, :], W_d[0:1, 0:D])
        nc.sync.dma_start(W_sb2[:, 1, :], W_d[0:1, 2 * D : 3 * D])
        wc_f32 = const.tile([128, 2, 1], f32, tag="wc_f32")
        nc.sync.dma_start(
            wc_f32[:], W_d[0, D : 2 * D].rearrange("(k p o) -> p k o", p=128, o=1)
        )
        Q_all = const.tile([LQ, BL, D], f32, tag="Q_all")
        nc.sync.dma_start(Q_all[:], Q_d.rearrange("b j d -> j b d"))

        # scalar ring: per-batch C loads, back to back (4KB lines)
        C_all = const.tile([128, BL, 4, D], f32, tag="C_all")
        nc.sync.dma_start(
            C_all[:, 0], C_d[0].rearrange("(p t) d -> p t d", t=4)
        )
        for b in range(1, BL):
            nc.scalar.dma_start(
                C_all[:, b], C_d[b].rearrange("(p t) d -> p t d", t=4)
            )

        # ---- constants ----
        ident = const.tile([128, 128], bf16, tag="ident")
        masks.make_identity(nc, ident[:])

        # broadcast wq/wi rows to 64 partitions via K=1 matmul with ones
        ones_row = const.tile([1, LQ], f32, tag="ones_row")
        nc.vector.memset(ones_row[:], 1.0)
        wb_ps = ps_si.tile([LQ, 2, D], f32, tag="si")
        nc.tensor.matmul(wb_ps[:], ones_row[:], W_sb2[:], start=True, stop=True)
        wqi = const.tile([LQ, 2, D], f32, tag="wqi")
        nc.scalar.copy(wqi[:], wb_ps[:])
        wq_b = wqi[:, 0, :]  # [64, 256] rows = wq
        wi_b = wqi[:, 1, :]  # [64, 256] rows = wi

        # persistent slotted bf16 tiles: the ones columns are written once,
        # casts only rewrite cols 0:256 each time a slot is reused
        C_bfs = const.tile([128, NSLOT, 4, D + 1], bf16, tag="C_bfs")
        nc.gpsimd.memset(C_bfs[:, :, :, D : D + 1], 1.0)
        Q_bfs = const.tile([LQ, NSLOT, D + 1], bf16, tag="Q_bfs")
        nc.gpsimd.memset(Q_bfs[:, :, D : D + 1], 1.0)

        st = {}  # per-batch tiles passed from phase A to phase B

        def phase_a(b):
            s = b % NSLOT
            Cb = C_all[:, b]  # [128, 4, 256] f32
            Qb = Q_all[:, b, :]  # [64, 256] f32
            C_bf = C_bfs[:, s]  # [128, 4, 257] bf16
            Q_bf = Q_bfs[:, s]  # [64, 257] bf16

            OUT = stg.tile([128, 4, 4 * D], f32, tag="OUT")

            # C_bf cast in halves on two engines so the PE transposes can
            # start as soon as the first half lands
            nc.vector.tensor_copy(C_bf[:, 0:2, 0:D], Cb[:, 0:2, :])
            nc.scalar.copy(C_bf[:, 2:4, 0:D], Cb[:, 2:4, :])

            # output block 0 = C (copied into the staging tile)
            nc.gpsimd.tensor_copy(OUT[:, 0:2, 0:D], Cb[:, 0:2, :])
            nc.scalar.copy(OUT[:, 2:4, 0:D], Cb[:, 2:4, :])

            nc.scalar.copy(Q_bf[:, 0:D], Qb)

            # Q' = Q * wi (bf16); s_q = rowsum(Q * wq) fused into one DVE op
            QP_bf = sb.tile([LQ, D], bf16, tag="QP_bf")
            nc.gpsimd.tensor_mul(QP_bf[:], Qb, wi_b)
            scr = sb.tile([LQ, D], bf16, tag="scr")
            s_q = sb.tile([LQ, 1], f32, tag="s_q")
            nc.vector.scalar_tensor_tensor(
                scr[:], Qb, 1.0, wq_b, op0=MUL, op1=MUL, accum_out=s_q[:]
            )

            # ---- transposes (PE) ----
            # tq: Q'^T -> [128, 2*64]; QW = [Q'^T_k | wc_k] [128, 2, 65]
            tq = ps_mm.tile([128, 128], bf16, tag="mm")
            for k in range(2):
                nc.tensor.transpose(
                    tq[:, ts(k, 64)], QP_bf[:, ts(k, 128)], ident[0:LQ, 0:LQ]
                )
            QW = sb.tile([128, 2, 65], bf16, tag="QW")
            nc.vector.tensor_copy(
                QW[:, :, 0:64], tq[:].rearrange("p (k j) -> p k j", k=2)
            )
            nc.vector.tensor_copy(QW[:, :, 64:65], wc_f32[:])

            # tc: C^T -> CT [128, 2, 512] (k = d-tile, free position t*128+p
            # corresponds to row i = 4p + t; consistent everywhere below)
            tcp = ps_tc.tile([128, 2, 512], bf16, tag="tcp")
            for t in range(4):
                for k in range(2):
                    nc.tensor.transpose(
                        tcp[:, k, ts(t, 128)], C_bf[:, t, ts(k, 128)], ident[:]
                    )
            CT = sb.tile([128, 2, 512], bf16, tag="CT")
            nc.vector.tensor_copy(CT[:, 0, :], tcp[:, 0, :])
            nc.scalar.copy(CT[:, 1, :], tcp[:, 1, :])

            st[b] = (C_bf, Q_bf, s_q, QW, CT, OUT)

        def phase_b(b):
            C_bf, Q_bf, s_q, QW, CT, OUT = st.pop(b)

            # ---- M1T: s_i^T [65, 512] (row 64 = s_c^T, unused) ----
            si_T = ps_si.tile([65, 512], f32, tag="si")
            for k in range(2):
                nc.tensor.matmul(
                    si_T[:], QW[:, k, :], CT[:, k, :], start=(k == 0), stop=(k == 1)
                )
            # E1_T = exp(s_i^T + s_q) (bf16)  [64, 512]
            E1_T = sb.tile([LQ, 512], bf16, tag="E1_T")
            nc.scalar.activation(E1_T[:], si_T[0:LQ, :], AF.Exp, bias=s_q[:])

            # ---- M1': s_i natural [128, 4, 65] (col 64 = s_c) ----
            si_n = ps_si.tile([128, 4, 65], f32, tag="si")
            for t in range(4):
                for k in range(2):
                    nc.tensor.matmul(
                        si_n[:, t, :],
                        CT[:, k, ts(t, 128)],
                        QW[:, k, :],
                        start=(k == 0),
                        stop=(k == 1),
                    )
            sc = sb.tile([128, 4, 1], f32, tag="sc")
            nc.vector.tensor_copy(sc[:], si_n[:, :, 64:65])
            # E2 = exp(s_i + s_c) (bf16)  [128, 4, 64]
            E2 = sb.tile([128, 4, 64], bf16, tag="E2")
            for t in range(4):
                nc.scalar.activation(
                    E2[:, t, :], si_n[:, t, 0:64], AF.Exp, bias=sc[:, t, :]
                )

            # ---- M3: P_C = E2^T @ [C|1] -> [64, 257] (col 256 = r2) ----
            pc = ps_mm.tile([LQ, D + 1], f32, tag="mm")
            for t in range(4):
                nc.tensor.matmul(
                    pc[:], E2[:, t, :], C_bf[:, t, :], start=(t == 0), stop=(t == 3)
                )
            rr2 = sb.tile([LQ, 1], f32, tag="rr2")
            nc.vector.reciprocal(rr2[:], pc[:, D : D + 1])
            C2_bf = sb.tile([LQ, D], bf16, tag="C2_bf")
            nc.vector.tensor_scalar_mul(C2_bf[:], pc[:, 0:D], rr2[:])

            # ---- M2: P_A[t] = E1 @ [Q|1] -> [128, 257] (col 256 = r1) ----
            # A block = P_A/r1; C*A block = (P_A*rr1)*C fused in one DVE op.
            # gpsimd cannot read PSUM: t=2,3 go PSUM->SBUF via ACT, then
            # gpsimd multiplies in SBUF.
            rr1 = sb.tile([128, 4, 1], f32, tag="rr1")
            for t in range(4):
                pa = ps_mm.tile([128, D + 1], f32, tag="mm")
                nc.tensor.matmul(
                    pa[:], E1_T[:, ts(t, 128)], Q_bf[:], start=True, stop=True
                )
                nc.vector.reciprocal(rr1[:, t, :], pa[:, D : D + 1])
                if t < 2:
                    nc.vector.tensor_scalar_mul(
                        OUT[:, t, D : 2 * D], pa[:, 0:D], rr1[:, t, :]
                    )
                    nc.vector.scalar_tensor_tensor(
                        OUT[:, t, 2 * D : 3 * D],
                        pa[:, 0:D],
                        rr1[:, t, :],
                        C_bf[:, t, 0:D],
                        op0=MUL,
                        op1=MUL,
                    )
                else:
                    nc.scalar.mul(OUT[:, t, D : 2 * D], pa[:, 0:D], rr1[:, t, :])
                    nc.gpsimd.tensor_mul(
                        OUT[:, t, 2 * D : 3 * D],
                        OUT[:, t, D : 2 * D],
                        C_bf[:, t, 0:D],
                    )

            # ---- M4: P_B[t] = E1 @ C2; C*Bm = (P_B*rr1)*C fused ----
            Bm_tmp = sb.tile([128, 2, D], f32, tag="Bm_tmp")
            for th in range(2):
                pb = ps_mm.tile([128, 2, D], f32, tag="mm")
                for h in range(2):
                    t = th * 2 + h
                    nc.tensor.matmul(
                        pb[:, h, :], E1_T[:, ts(t, 128)], C2_bf[:], start=True, stop=True
                    )
                    if t < 2:
                        nc.vector.scalar_tensor_tensor(
                            OUT[:, t, 3 * D : 4 * D],
                            pb[:, h, :],
                            rr1[:, t, :],
                            C_bf[:, t, 0:D],
                            op0=MUL,
                            op1=MUL,
                        )
                    else:
                        nc.scalar.mul(Bm_tmp[:, h, :], pb[:, h, :], rr1[:, t, :])
                        nc.gpsimd.tensor_mul(
                            OUT[:, t, 3 * D : 4 * D],
                            Bm_tmp[:, h, :],
                            C_bf[:, t, 0:D],
                        )

            # ---- single 2MB store: per-partition DRAM span is 16KB ----
            nc.sync.dma_start(
                out_d[b].rearrange("(p t) dd -> p t dd", t=4), OUT[:]
            )

        # phase A runs LOOKAHEAD batches ahead of phase B so the in-order PE
        # stream always has transpose work and stores overlap later compute
        for b in range(min(LOOKAHEAD, BL)):
            phase_a(b)
        for b in range(BL):
            if b + LOOKAHEAD < BL:
                phase_a(b + LOOKAHEAD)
            phase_b(b)

    nc.compile()
    return nc


def _get_nc():
    global _NC_CACHE
    if _NC_CACHE is None:
        _NC_CACHE = _build_nc()
    return _NC_CACHE


def _make_in_maps(contex, question, W_weight):
    contex = np.asarray(contex, dtype=np.float32)
    question = np.asarray(question, dtype=np.float32)
    W_weight = np.asarray(W_weight, dtype=np.float32)
    in_maps = []
    for c in range(NCORES):
        sl = slice(c * BL, (c + 1) * BL)
        in_maps.append(
            {
                "contex": np.ascontiguousarray(contex[sl]),
                "question": np.ascontiguousarray(question[sl]),
                "W_weight": W_weight,
            }
        )
    return in_maps


def run_spmd(contex, question, W_weight, trace=False, tmpdir=None):
    """Returns (out [64,512,1024] f32, exec_time_ns or None)."""
    from concourse.bass_utils import run_bass_kernel_spmd

    nc = _get_nc()
    in_maps = _make_in_maps(contex, question, W_weight)
    res = run_bass_kernel_spmd(
        nc, in_maps, list(range(NCORES)), trace=trace, tmpdir=tmpdir
    )
    out = np.concatenate([res.results[c]["out"] for c in range(NCORES)], axis=0)
    return out, res.exec_time_ns


def kernel(contex, question, W_weight, W_bias=None, **_unused):
    # W_bias provably has no effect on the output (it is a constant shift
    # inside both softmaxes), so it is not shipped to the device.
    out, _ = run_spmd(contex, question, W_weight, trace=False)
    return out


# revision 6
# speedup vs baseline: 1.0814x; 1.0814x over previous
"""CQAttention (BiDAF-style context-query attention) on 8 TRN2 NeuronCores.

Full shapes: contex [64, 512, 256], question [64, 64, 256],
W_weight [1, 768], W_bias [1] -> out [64, 512, 1024].

Sharding: pure data-parallel over batch, 8 batches per core.

Math notes (per batch, C=[512,256], Q=[64,256], w=[wq|wc|wi]):
  S[i,j] = sum_d C[i,d]*wi[d]*Q[j,d] + C[i].wc + Q[j].wq + b
  S1 = softmax_j(S), S2 = softmax_i(S)
  - b drops out of both softmaxes; s_c drops out of S1; s_q drops out of S2.
  - E1 = exp(s_i + s_q[j]), r1[i] = sum_j E1;  S1 = E1/r1
  - E2 = exp(s_i + s_c[i]), r2[j] = sum_i E2;  S2 = E2/r2
  - A  = S1 @ Q = (E1 @ Q)/r1
  - Bm = (S1 @ S2^T) @ C = S1 @ (S2^T @ C) = (E1 @ C2)/r1, C2 = (E2^T @ C)/r2
  r1/r2 are obtained for free as ones-columns appended to the matmul rhs.
  out = [C | A | C*A | C*Bm]

DMA design:
  - context rows are mapped i = 4p + t (partition-major): C loads move
    4KB-contiguous lines; the merged [A|C*A|C*Bm] store moves 3KB lines.
  - ALL input DMAs are issued up front (before any compute is emitted) into
    persistent tiles, so no load ever queues behind compute on its issuing
    engine.  C batch 0 rides the sync ring in parallel with Q on the
    scalar ring so batch 0 can start ASAP.
  - The C output block is stored straight from the persistent C_all input
    tile on the scalar ring (idle after the loads drain) — no copy.
  - The other three blocks are assembled in one [128, 4, 768] staging tile
    and shipped as a single 1.5MB store on the sync ring.

Emission is a 4-stage software pipeline; each "step" emits, in this order,
  S4(b-3): M2/M4 + normalization/products + store   (uses E1,C2 from b-3)
  S3(b-2): M3 + 1/r2 + C2
  S2(b-1): M1T/M1' + exps
  S1(b):   casts, Q'*wi, s_q, PE transposes of C
Reverse-stage order puts instructions whose inputs are oldest (most likely
ready) at the head of every engine queue, which keeps the in-order engines
from head-of-line blocking on same-step dependency chains.
"""

import numpy as np

B, LC, LQ, D = 64, 512, 64, 256
NCORES = 8
BL = B // NCORES  # batches per core
NSLOT = 5

_NC_CACHE = None


def _build_nc():
    import concourse.bass as bass
    import concourse.mybir as mybir
    from concourse import bacc
    from concourse import masks
    from concourse import tile
    from contextlib import ExitStack

    f32 = mybir.dt.float32
    bf16 = mybir.dt.bfloat16
    AF = mybir.ActivationFunctionType
    MUL = mybir.AluOpType.mult
    ts = bass.ts

    nc = bacc.Bacc("TRN2", target_bir_lowering=False, debug=False)
    C_d = nc.dram_tensor("contex", [BL, LC, D], f32, kind="ExternalInput")
    Q_d = nc.dram_tensor("question", [BL, LQ, D], f32, kind="ExternalInput")
    W_d = nc.dram_tensor("W_weight", [1, 3 * D], f32, kind="ExternalInput")
    out_d = nc.dram_tensor("out", [BL, LC, 4 * D], f32, kind="ExternalOutput")

    with tile.TileContext(nc) as tc, ExitStack() as ctx:
        const = ctx.enter_context(tc.tile_pool(name="const", bufs=1))
        sb = ctx.enter_context(tc.tile_pool(name="sb", bufs=NSLOT))
        stg = ctx.enter_context(tc.tile_pool(name="stg", bufs=3))
        ps_tc = ctx.enter_context(tc.tile_pool(name="ps_tc", bufs=2, space="PSUM"))
        ps_si = ctx.enter_context(tc.tile_pool(name="ps_si", bufs=2, space="PSUM"))
        ps_mm = ctx.enter_context(tc.tile_pool(name="ps_mm", bufs=4, space="PSUM"))

        # ---- all input DMAs, issued before any compute exists ----
        # sync ring: weights + C batch 0 (the main stores start much later)
        W_sb2 = const.tile([1, 2, D], f32, tag="W_sb2")
        nc.sync.dma_start(W_sb2[:, 0, :], W_d[0:1, 0:D])
        nc.sync.dma_start(W_sb2[:, 1, :], W_d[0:1, 2 * D : 3 * D])
        wc_f32 = const.tile([128, 2, 1], f32, tag="wc_f32")
        nc.sync.dma_start(
            wc_f32[:], W_d[0, D : 2 * D].rearrange("(k p o) -> p k o", p=128, o=1)
        )
        C_all = const.tile([128, BL, 4, D], f32, tag="C_all")
        nc.sync.dma_start(C_all[:, 0], C_d[0].rearrange("(p t) d -> p t d", t=4))

        # scalar ring: Q + the remaining C batches, back to back (4KB lines)
        Q_all = const.tile([LQ, BL, D], f32, tag="Q_all")
        nc.scalar.dma_start(Q_all[:], Q_d.rearrange("b j d -> j b d"))
        for b in range(1, BL):
            nc.scalar.dma_start(
                C_all[:, b], C_d[b].rearrange("(p t) d -> p t d", t=4)
            )

        # ---- constants ----
        ident = const.tile([128, 128], bf16, tag="ident")
        masks.make_identity(nc, ident[:])

        # broadcast wq/wi rows to 64 partitions via K=1 matmul with ones
        ones_row = const.tile([1, LQ], f32, tag="ones_row")
        nc.vector.memset(ones_row[:], 1.0)
        wb_ps = ps_si.tile([LQ, 2, D], f32, tag="si")
        nc.tensor.matmul(wb_ps[:], ones_row[:], W_sb2[:], start=True, stop=True)
        wqi = const.tile([LQ, 2, D], f32, tag="wqi")
        nc.scalar.copy(wqi[:], wb_ps[:])
        wq_b = wqi[:, 0, :]  # [64, 256] rows = wq
        wi_b = wqi[:, 1, :]  # [64, 256] rows = wi

        # persistent slotted bf16 tiles: the ones columns are written once,
        # casts only rewrite cols 0:256 each time a slot is reused
        C_bfs = const.tile([128, NSLOT, 4, D + 1], bf16, tag="C_bfs")
        nc.gpsimd.memset(C_bfs[:, :, :, D : D + 1], 1.0)
        Q_bfs = const.tile([LQ, NSLOT, D + 1], bf16, tag="Q_bfs")
        nc.gpsimd.memset(Q_bfs[:, :, D : D + 1], 1.0)

        st1, st2, st3 = {}, {}, {}  # stage-boundary state, keyed by batch

        def stage1(b):
            s = b % NSLOT
            Cb = C_all[:, b]  # [128, 4, 256] f32
            Qb = Q_all[:, b, :]  # [64, 256] f32
            C_bf = C_bfs[:, s]  # [128, 4, 257] bf16
            Q_bf = Q_bfs[:, s]  # [64, 257] bf16

            # ship output block 0 = C straight from the input tile
            # (scalar ring; it drains after the input loads finish)
            nc.scalar.dma_start(
                out_d[b].rearrange("(p t) dd -> p t dd", t=4)[:, :, 0:D], Cb
            )

            # C_bf cast in halves on two engines
            nc.vector.tensor_copy(C_bf[:, 0:2, 0:D], Cb[:, 0:2, :])
            nc.scalar.copy(C_bf[:, 2:4, 0:D], Cb[:, 2:4, :])
            nc.scalar.copy(Q_bf[:, 0:D], Qb)

            # Q' = Q * wi (bf16); s_q = rowsum(Q * wq) fused into one DVE op
            QP_bf = sb.tile([LQ, D], bf16, tag="QP_bf")
            nc.gpsimd.tensor_mul(QP_bf[:], Qb, wi_b)
            scr = sb.tile([LQ, D], bf16, tag="scr")
            s_q = sb.tile([LQ, 1], f32, tag="s_q")
            nc.vector.scalar_tensor_tensor(
                scr[:], Qb, 1.0, wq_b, op0=MUL, op1=MUL, accum_out=s_q[:]
            )

            # tq: Q'^T -> [128, 2*64]; QW = [Q'^T_k | wc_k] [128, 2, 65]
            tq = ps_mm.tile([128, 128], bf16, tag="mm")
            for k in range(2):
                nc.tensor.transpose(
                    tq[:, ts(k, 64)], QP_bf[:, ts(k, 128)], ident[0:LQ, 0:LQ]
                )
            QW = sb.tile([128, 2, 65], bf16, tag="QW")
            nc.vector.tensor_copy(
                QW[:, :, 0:64], tq[:].rearrange("p (k j) -> p k j", k=2)
            )
            nc.vector.tensor_copy(QW[:, :, 64:65], wc_f32[:])

            # tc: C^T -> CT [128, 2, 512] (k = d-tile, free position t*128+p
            # corresponds to row i = 4p + t; consistent everywhere below)
            tcp = ps_tc.tile([128, 2, 512], bf16, tag="tcp")
            for t in range(4):
                for k in range(2):
                    nc.tensor.transpose(
                        tcp[:, k, ts(t, 128)], C_bf[:, t, ts(k, 128)], ident[:]
                    )
            CT = sb.tile([128, 2, 512], bf16, tag="CT")
            nc.vector.tensor_copy(CT[:, 0, :], tcp[:, 0, :])
            nc.scalar.copy(CT[:, 1, :], tcp[:, 1, :])

            st1[b] = (C_bf, Q_bf, s_q, QW, CT)

        def stage2(b):
            C_bf, Q_bf, s_q, QW, CT = st1.pop(b)

            # ---- M1T: s_i^T [65, 512] (row 64 = s_c^T, unused) ----
            si_T = ps_si.tile([65, 512], f32, tag="si")
            for k in range(2):
                nc.tensor.matmul(
                    si_T[:], QW[:, k, :], CT[:, k, :], start=(k == 0), stop=(k == 1)
                )
            # E1_T = exp(s_i^T + s_q) (bf16)  [64, 512]
            E1_T = sb.tile([LQ, 512], bf16, tag="E1_T")
            nc.scalar.activation(E1_T[:], si_T[0:LQ, :], AF.Exp, bias=s_q[:])

            # ---- M1': s_i natural [128, 4, 65] (col 64 = s_c) ----
            si_n = ps_si.tile([128, 4, 65], f32, tag="si")
            for t in range(4):
                for k in range(2):
                    nc.tensor.matmul(
                        si_n[:, t, :],
                        CT[:, k, ts(t, 128)],
                        QW[:, k, :],
                        start=(k == 0),
                        stop=(k == 1),
                    )
            sc = sb.tile([128, 4, 1], f32, tag="sc")
            nc.scalar.copy(sc[:], si_n[:, :, 64:65])
            # E2 = exp(s_i + s_c) (bf16)  [128, 4, 64]
            E2 = sb.tile([128, 4, 64], bf16, tag="E2")
            for t in range(4):
                nc.scalar.activation(
                    E2[:, t, :], si_n[:, t, 0:64], AF.Exp, bias=sc[:, t, :]
                )
            st2[b] = (C_bf, Q_bf, E1_T, E2)

        def stage3(b):
            C_bf, Q_bf, E1_T, E2 = st2.pop(b)

            # ---- M3: P_C = E2^T @ [C|1] -> [64, 257] (col 256 = r2) ----
            pc = ps_mm.tile([LQ, D + 1], f32, tag="mm")
            for t in range(4):
                nc.tensor.matmul(
                    pc[:], E2[:, t, :], C_bf[:, t, :], start=(t == 0), stop=(t == 3)
                )
            rr2 = sb.tile([LQ, 1], f32, tag="rr2")
            nc.vector.reciprocal(rr2[:], pc[:, D : D + 1])
            C2_bf = sb.tile([LQ, D], bf16, tag="C2_bf")
            nc.scalar.mul(C2_bf[:], pc[:, 0:D], rr2[:])
            st3[b] = (C_bf, Q_bf, E1_T, C2_bf)

        def stage4(b):
            C_bf, Q_bf, E1_T, C2_bf = st3.pop(b)
            OUT = stg.tile([128, 4, 3 * D], f32, tag="OUT")

            # ---- M2: P_A[t] = E1 @ [Q|1] -> [128, 257] (col 256 = r1) ----
            # A block = P_A*rr1; C*A block = (P_A*rr1)*C fused in one DVE op.
            # gpsimd cannot read PSUM, so its products read SBUF results.
            rr1 = sb.tile([128, 4, 1], f32, tag="rr1")
            for t in range(4):
                pa = ps_mm.tile([128, D + 1], f32, tag="mm")
                nc.tensor.matmul(
                    pa[:], E1_T[:, ts(t, 128)], Q_bf[:], start=True, stop=True
                )
                nc.vector.reciprocal(rr1[:, t, :], pa[:, D : D + 1])
                if t < 2:
                    nc.vector.tensor_scalar_mul(
                        OUT[:, t, 0:D], pa[:, 0:D], rr1[:, t, :]
                    )
                else:
                    nc.scalar.mul(OUT[:, t, 0:D], pa[:, 0:D], rr1[:, t, :])
                if t < 3:
                    nc.vector.scalar_tensor_tensor(
                        OUT[:, t, D : 2 * D],
                        pa[:, 0:D],
                        rr1[:, t, :],
                        C_bf[:, t, 0:D],
                        op0=MUL,
                        op1=MUL,
                    )
                else:
                    nc.gpsimd.tensor_mul(
                        OUT[:, t, D : 2 * D], OUT[:, t, 0:D], C_bf[:, t, 0:D]
                    )

            # ---- M4: P_B[t] = E1 @ C2; C*Bm = (P_B*rr1)*C fused ----
            Bm_tmp = sb.tile([128, 2, D], f32, tag="Bm_tmp")
            for th in range(2):
                pb = ps_mm.tile([128, 2, D], f32, tag="mm")
                for h in range(2):
                    t = th * 2 + h
                    nc.tensor.matmul(
                        pb[:, h, :], E1_T[:, ts(t, 128)], C2_bf[:], start=True, stop=True
                    )
                    if t < 2:
                        nc.vector.scalar_tensor_tensor(
                            OUT[:, t, 2 * D : 3 * D],
                            pb[:, h, :],
                            rr1[:, t, :],
                            C_bf[:, t, 0:D],
                            op0=MUL,
                            op1=MUL,
                        )
                    else:
                        nc.scalar.mul(Bm_tmp[:, h, :], pb[:, h, :], rr1[:, t, :])
                        nc.gpsimd.tensor_mul(
                            OUT[:, t, 2 * D : 3 * D],
                            Bm_tmp[:, h, :],
                            C_bf[:, t, 0:D],
                        )

            # ---- single 1.5MB store of [A | C*A | C*Bm] (sync ring) ----
            nc.sync.dma_start(
                out_d[b].rearrange("(p t) dd -> p t dd", t=4)[:, :, D : 4 * D],
                OUT[:],
            )

        # 4-stage software pipeline, reverse-stage emission within a step
        for step in range(BL + 3):
            if step >= 3:
                stage4(step - 3)
            if 2 <= step < BL + 2:
                stage3(step - 2)
            if 1 <= step < BL + 1:
                stage2(step - 1)
            if step < BL:
                stage1(step)

    nc.compile()
    return nc


def _get_nc():
    global _NC_CACHE
    if _NC_CACHE is None:
        _NC_CACHE = _build_nc()
    return _NC_CACHE


def _make_in_maps(contex, question, W_weight):
    contex = np.asarray(contex, dtype=np.float32)
    question = np.asarray(question, dtype=np.float32)
    W_weight = np.asarray(W_weight, dtype=np.float32)
    in_maps = []
    for c in range(NCORES):
        sl = slice(c * BL, (c + 1) * BL)
        in_maps.append(
            {
                "contex": np.ascontiguousarray(contex[sl]),
                "question": np.ascontiguousarray(question[sl]),
                "W_weight": W_weight,
            }
        )
    return in_maps


def run_spmd(contex, question, W_weight, trace=False, tmpdir=None):
    """Returns (out [64,512,1024] f32, exec_time_ns or None)."""
    from concourse.bass_utils import run_bass_kernel_spmd

    nc = _get_nc()
    in_maps = _make_in_maps(contex, question, W_weight)
    res = run_bass_kernel_spmd(
        nc, in_maps, list(range(NCORES)), trace=trace, tmpdir=tmpdir
    )
    out = np.concatenate([res.results[c]["out"] for c in range(NCORES)], axis=0)
    return out, res.exec_time_ns


def kernel(contex, question, W_weight, W_bias=None, **_unused):
    # W_bias provably has no effect on the output (it is a constant shift
    # inside both softmaxes), so it is not shipped to the device.
    out, _ = run_spmd(contex, question, W_weight, trace=False)
    return out


# revision 10
# speedup vs baseline: 1.1128x; 1.0291x over previous
"""CQAttention (BiDAF-style context-query attention) on 8 TRN2 NeuronCores.

Full shapes: contex [64, 512, 256], question [64, 64, 256],
W_weight [1, 768], W_bias [1] -> out [64, 512, 1024].

Sharding: pure data-parallel over batch, 8 batches per core.

Math notes (per batch, C=[512,256], Q=[64,256], w=[wq|wc|wi]):
  S[i,j] = sum_d C[i,d]*wi[d]*Q[j,d] + C[i].wc + Q[j].wq + b
  S1 = softmax_j(S), S2 = softmax_i(S)
  - b drops out of both softmaxes; s_c drops out of S1; s_q drops out of S2.
  - E1 = exp(s_i + s_q[j]), r1[i] = sum_j E1;  S1 = E1/r1
  - E2 = exp(s_i + s_c[i]), r2[j] = sum_i E2;  S2 = E2/r2
  - A  = S1 @ Q = (E1 @ Q)/r1
  - Bm = (S1 @ S2^T) @ C = S1 @ (S2^T @ C) = (E1 @ C2)/r1, C2 = (E2^T @ C)/r2
  r1/r2 are obtained for free as ones-columns appended to the matmul rhs.
  out = [C | A | C*A | C*Bm]

DMA design:
  - context rows are mapped i = 4p + t (partition-major): C loads move
    4KB-contiguous lines; the merged [A|C*A|C*Bm] store moves 3KB lines.
  - ALL input DMAs are issued up front (before any compute is emitted) into
    persistent tiles, so no load ever queues behind compute on its issuing
    engine.  C batch 0 rides the sync ring in parallel with Q on the
    scalar ring so batch 0 can start ASAP.
  - The C output block is stored straight from the persistent C_all input
    tile on the scalar ring (idle after the loads drain) — no copy.
  - The other three blocks are assembled in one [128, 4, 768] staging tile
    and shipped as a single 1.5MB store on the sync ring.

Emission is a 4-stage software pipeline; each "step" emits, in this order,
  S4(b-3): M2/M4 + normalization/products + store   (uses E1,C2 from b-3)
  S3(b-2): M3 + 1/r2 + C2
  S2(b-1): M1T/M1' + exps
  S1(b):   casts, Q'*wi, s_q, PE transposes of C
Reverse-stage order puts instructions whose inputs are oldest (most likely
ready) at the head of every engine queue, which keeps the in-order engines
from head-of-line blocking on same-step dependency chains.
"""

import numpy as np

B, LC, LQ, D = 64, 512, 64, 256
NCORES = 8
BL = B // NCORES  # batches per core
NSLOT = 5

_NC_CACHE = None


def _build_nc():
    import concourse.bass as bass
    import concourse.mybir as mybir
    from concourse import bacc
    from concourse import masks
    from concourse import tile
    from contextlib import ExitStack

    f32 = mybir.dt.float32
    bf16 = mybir.dt.bfloat16
    AF = mybir.ActivationFunctionType
    MUL = mybir.AluOpType.mult
    ts = bass.ts

    nc = bacc.Bacc("TRN2", target_bir_lowering=False, debug=False)
    C_d = nc.dram_tensor("contex", [BL, LC, D], f32, kind="ExternalInput")
    Q_d = nc.dram_tensor("question", [BL, LQ, D], f32, kind="ExternalInput")
    W_d = nc.dram_tensor("W_weight", [1, 3 * D], f32, kind="ExternalInput")
    out_d = nc.dram_tensor("out", [BL, LC, 4 * D], f32, kind="ExternalOutput")

    with tile.TileContext(nc) as tc, ExitStack() as ctx:
        const = ctx.enter_context(tc.tile_pool(name="const", bufs=1))
        sb = ctx.enter_context(tc.tile_pool(name="sb", bufs=NSLOT))
        stg = ctx.enter_context(tc.tile_pool(name="stg", bufs=3))
        ps_tc = ctx.enter_context(tc.tile_pool(name="ps_tc", bufs=2, space="PSUM"))
        ps_si = ctx.enter_context(tc.tile_pool(name="ps_si", bufs=2, space="PSUM"))
        ps_mm = ctx.enter_context(tc.tile_pool(name="ps_mm", bufs=4, space="PSUM"))

        # ---- all input DMAs, issued before any compute exists ----
        # sync ring: C batch 0 first, then weights (main stores start later)
        C_all = const.tile([128, BL, 4, D], f32, tag="C_all")
        nc.sync.dma_start(C_all[:, 0], C_d[0].rearrange("(p t) d -> p t d", t=4))
        W_sb2 = const.tile([1, 2, D], f32, tag="W_sb2")
        nc.sync.dma_start(W_sb2[:, 0, :], W_d[0:1, 0:D])
        nc.sync.dma_start(W_sb2[:, 1, :], W_d[0:1, 2 * D : 3 * D])
        wc_f32 = const.tile([128, 2, 1], f32, tag="wc_f32")
        nc.sync.dma_start(
            wc_f32[:], W_d[0, D : 2 * D].rearrange("(k p o) -> p k o", p=128, o=1)
        )

        # scalar ring: Q + the remaining C batches, back to back (4KB lines)
        Q_all = const.tile([LQ, BL, D], f32, tag="Q_all")
        nc.scalar.dma_start(Q_all[:], Q_d.rearrange("b j d -> j b d"))
        for b in range(1, BL):
            nc.scalar.dma_start(
                C_all[:, b], C_d[b].rearrange("(p t) d -> p t d", t=4)
            )

        # ---- constants ----
        ident = const.tile([128, 128], bf16, tag="ident")
        masks.make_identity(nc, ident[:])

        # broadcast wq/wi rows to 64 partitions via K=1 matmul with ones
        ones_row = const.tile([1, LQ], f32, tag="ones_row")
        nc.vector.memset(ones_row[:], 1.0)
        wb_ps = ps_si.tile([LQ, 2, D], f32, tag="si")
        nc.tensor.matmul(wb_ps[:], ones_row[:], W_sb2[:], start=True, stop=True)
        wqi = const.tile([LQ, 2, D], f32, tag="wqi")
        nc.scalar.copy(wqi[:], wb_ps[:])
        wq_b = wqi[:, 0, :]  # [64, 256] rows = wq
        wi_b = wqi[:, 1, :]  # [64, 256] rows = wi

        # persistent slotted bf16 tiles: the ones columns are written once,
        # casts only rewrite cols 0:256 each time a slot is reused
        C_bfs = const.tile([128, NSLOT, 4, D + 1], bf16, tag="C_bfs")
        nc.gpsimd.memset(C_bfs[:, :, :, D : D + 1], 1.0)
        Q_bfs = const.tile([LQ, NSLOT, D + 1], bf16, tag="Q_bfs")
        nc.gpsimd.memset(Q_bfs[:, :, D : D + 1], 1.0)

        st1, st2, st3 = {}, {}, {}  # stage-boundary state, keyed by batch

        def stage1(b):
            s = b % NSLOT
            Cb = C_all[:, b]  # [128, 4, 256] f32
            Qb = Q_all[:, b, :]  # [64, 256] f32
            C_bf = C_bfs[:, s]  # [128, 4, 257] bf16
            Q_bf = Q_bfs[:, s]  # [64, 257] bf16

            # ship output block 0 = C straight from the input tile
            # (scalar ring; it drains after the input loads finish)
            nc.scalar.dma_start(
                out_d[b].rearrange("(p t) dd -> p t dd", t=4)[:, :, 0:D], Cb
            )

            # C_bf cast in halves on two engines
            nc.vector.tensor_copy(C_bf[:, 0:2, 0:D], Cb[:, 0:2, :])
            nc.scalar.copy(C_bf[:, 2:4, 0:D], Cb[:, 2:4, :])
            nc.gpsimd.tensor_copy(Q_bf[:, 0:D], Qb)

            # Q' = Q * wi (bf16); s_q = rowsum(Q * wq) fused into one DVE op
            QP_bf = sb.tile([LQ, D], bf16, tag="QP_bf")
            nc.gpsimd.tensor_mul(QP_bf[:], Qb, wi_b)
            scr = sb.tile([LQ, D], bf16, tag="scr")
            s_q = sb.tile([LQ, 1], f32, tag="s_q")
            nc.vector.scalar_tensor_tensor(
                scr[:], Qb, 1.0, wq_b, op0=MUL, op1=MUL, accum_out=s_q[:]
            )

            # tq: Q'^T -> [128, 2*64]; QW = [Q'^T_k | wc_k] [128, 2, 65]
            tq = ps_mm.tile([128, 128], bf16, tag="mm")
            for k in range(2):
                nc.tensor.transpose(
                    tq[:, ts(k, 64)], QP_bf[:, ts(k, 128)], ident[0:LQ, 0:LQ]
                )
            QW = sb.tile([128, 2, 65], bf16, tag="QW")
            nc.vector.tensor_copy(
                QW[:, :, 0:64], tq[:].rearrange("p (k j) -> p k j", k=2)
            )
            nc.vector.tensor_copy(QW[:, :, 64:65], wc_f32[:])

            # tc: C^T -> CT [128, 2, 512] (k = d-tile, free position t*128+p
            # corresponds to row i = 4p + t; consistent everywhere below)
            tcp = ps_tc.tile([128, 2, 512], bf16, tag="tcp")
            for t in range(4):
                for k in range(2):
                    nc.tensor.transpose(
                        tcp[:, k, ts(t, 128)], C_bf[:, t, ts(k, 128)], ident[:]
                    )
            CT = sb.tile([128, 2, 512], bf16, tag="CT")
            nc.scalar.copy(CT[:], tcp[:])

            st1[b] = (C_bf, Q_bf, s_q, QW, CT)

        def stage2(b):
            C_bf, Q_bf, s_q, QW, CT = st1.pop(b)

            # ---- M1T: s_i^T [65, 512] (row 64 = s_c^T, unused) ----
            si_T = ps_si.tile([65, 512], f32, tag="si")
            for k in range(2):
                nc.tensor.matmul(
                    si_T[:], QW[:, k, :], CT[:, k, :], start=(k == 0), stop=(k == 1)
                )
            # E1_T = exp(s_i^T + s_q) (bf16)  [64, 512]
            E1_T = sb.tile([LQ, 512], bf16, tag="E1_T")
            nc.scalar.activation(E1_T[:], si_T[0:LQ, :], AF.Exp, bias=s_q[:])

            # ---- M1': s_i natural [128, 4, 65] (col 64 = s_c) ----
            si_n = ps_si.tile([128, 4, 65], f32, tag="si")
            for t in range(4):
                for k in range(2):
                    nc.tensor.matmul(
                        si_n[:, t, :],
                        CT[:, k, ts(t, 128)],
                        QW[:, k, :],
                        start=(k == 0),
                        stop=(k == 1),
                    )
            # E2 = exp(s_i) plain, in ONE activation; the s_c bias is folded
            # into the M3 rhs instead: CS = exp(s_c[i]) * [C|1] row-scale,
            # since sum_i exp(si+sc)*X[i] == sum_i exp(si) * (exp(sc)*X[i]).
            E2 = sb.tile([128, 4, 64], bf16, tag="E2")
            nc.scalar.activation(E2[:], si_n[:, :, 0:64], AF.Exp)
            exp_sc = sb.tile([128, 4, 1], f32, tag="exp_sc")
            nc.scalar.activation(exp_sc[:], si_n[:, :, 64:65], AF.Exp)
            CS_bf = sb.tile([128, 4, D + 1], bf16, tag="CS_bf")
            cs_a, cs_b = bass.broadcast_tensor_aps(C_bf[:], exp_sc[:])
            nc.vector.tensor_mul(CS_bf[:], cs_a, cs_b)
            st2[b] = (C_bf, Q_bf, E1_T, E2, CS_bf)

        def stage3(b):
            C_bf, Q_bf, E1_T, E2, CS_bf = st2.pop(b)

            # ---- M3: P_C = E2^T @ CS -> [64, 257] (col 256 = r2) ----
            pc = ps_mm.tile([LQ, D + 1], f32, tag="mm")
            for t in range(4):
                nc.tensor.matmul(
                    pc[:], E2[:, t, :], CS_bf[:, t, :], start=(t == 0), stop=(t == 3)
                )
            rr2 = sb.tile([LQ, 1], f32, tag="rr2")
            nc.vector.reciprocal(rr2[:], pc[:, D : D + 1])
            C2_bf = sb.tile([LQ, D], bf16, tag="C2_bf")
            nc.scalar.mul(C2_bf[:], pc[:, 0:D], rr2[:])
            st3[b] = (C_bf, Q_bf, E1_T, C2_bf)

        def stage4(b):
            C_bf, Q_bf, E1_T, C2_bf = st3.pop(b)
            OUT = stg.tile([128, 4, 3 * D], f32, tag="OUT")

            # ---- M2: P_A[t] = E1 @ [Q|1] -> [128, 257] (col 256 = r1) ----
            # A block = P_A*rr1; C*A block = (P_A*rr1)*C fused in one DVE op.
            # gpsimd cannot read PSUM, so its products read SBUF results.
            rr1 = sb.tile([128, 4, 1], f32, tag="rr1")
            for t in range(4):
                pa = ps_mm.tile([128, D + 1], f32, tag="mm")
                nc.tensor.matmul(
                    pa[:], E1_T[:, ts(t, 128)], Q_bf[:], start=True, stop=True
                )
                nc.vector.reciprocal(rr1[:, t, :], pa[:, D : D + 1])
                if t < 2:
                    nc.vector.tensor_scalar_mul(
                        OUT[:, t, 0:D], pa[:, 0:D], rr1[:, t, :]
                    )
                else:
                    nc.scalar.mul(OUT[:, t, 0:D], pa[:, 0:D], rr1[:, t, :])
                if t < 3:
                    nc.vector.scalar_tensor_tensor(
                        OUT[:, t, D : 2 * D],
                        pa[:, 0:D],
                        rr1[:, t, :],
                        C_bf[:, t, 0:D],
                        op0=MUL,
                        op1=MUL,
                    )
                else:
                    nc.gpsimd.tensor_mul(
                        OUT[:, t, D : 2 * D], OUT[:, t, 0:D], C_bf[:, t, 0:D]
                    )

            # ---- M4: P_B[t] = E1 @ C2; C*Bm = (P_B*rr1)*C fused ----
            Bm_tmp = sb.tile([128, 2, D], f32, tag="Bm_tmp")
            for th in range(2):
                pb = ps_mm.tile([128, 2, D], f32, tag="mm")
                for h in range(2):
                    t = th * 2 + h
                    nc.tensor.matmul(
                        pb[:, h, :], E1_T[:, ts(t, 128)], C2_bf[:], start=True, stop=True
                    )
                    if t < 2:
                        nc.vector.scalar_tensor_tensor(
                            OUT[:, t, 2 * D : 3 * D],
                            pb[:, h, :],
                            rr1[:, t, :],
                            C_bf[:, t, 0:D],
                            op0=MUL,
                            op1=MUL,
                        )
                    else:
                        nc.scalar.mul(Bm_tmp[:, h, :], pb[:, h, :], rr1[:, t, :])
                        nc.gpsimd.tensor_mul(
                            OUT[:, t, 2 * D : 3 * D],
                            Bm_tmp[:, h, :],
                            C_bf[:, t, 0:D],
                        )

            # ---- single 1.5MB store of [A | C*A | C*Bm] (sync ring) ----
            nc.sync.dma_start(
                out_d[b].rearrange("(p t) dd -> p t dd", t=4)[:, :, D : 4 * D],
                OUT[:],
            )

        # 4-stage software pipeline, reverse-stage emission within a step
        for step in range(BL + 3):
            if step >= 3:
                stage4(step - 3)
            if 2 <= step < BL + 2:
                stage3(step - 2)
            if 1 <= step < BL + 1:
                stage2(step - 1)
            if step < BL:
                stage1(step)

    nc.compile()
    return nc


def _get_nc():
    global _NC_CACHE
    if _NC_CACHE is None:
        _NC_CACHE = _build_nc()
    return _NC_CACHE


def _make_in_maps(contex, question, W_weight):
    contex = np.asarray(contex, dtype=np.float32)
    question = np.asarray(question, dtype=np.float32)
    W_weight = np.asarray(W_weight, dtype=np.float32)
    in_maps = []
    for c in range(NCORES):
        sl = slice(c * BL, (c + 1) * BL)
        in_maps.append(
            {
                "contex": np.ascontiguousarray(contex[sl]),
                "question": np.ascontiguousarray(question[sl]),
                "W_weight": W_weight,
            }
        )
    return in_maps


def run_spmd(contex, question, W_weight, trace=False, tmpdir=None):
    """Returns (out [64,512,1024] f32, exec_time_ns or None)."""
    from concourse.bass_utils import run_bass_kernel_spmd

    nc = _get_nc()
    in_maps = _make_in_maps(contex, question, W_weight)
    res = run_bass_kernel_spmd(
        nc, in_maps, list(range(NCORES)), trace=trace, tmpdir=tmpdir
    )
    out = np.concatenate([res.results[c]["out"] for c in range(NCORES)], axis=0)
    return out, res.exec_time_ns


def kernel(contex, question, W_weight, W_bias=None, **_unused):
    # W_bias provably has no effect on the output (it is a constant shift
    # inside both softmaxes), so it is not shipped to the device.
    out, _ = run_spmd(contex, question, W_weight, trace=False)
    return out
